# revision 1
# baseline (speedup 1.0000x reference)
"""Trainium2 8-core kernel v3: RMSNorm -> QKV -> RoPE -> causal SDPA -> out-proj.

Sharding: core c = b*4 + g handles batch b (of 2) and heads 4g..4g+3 (of 16).
Host sums the 4 head-group partial out-projections per batch and adds b_o.

Structure (v3):
- RMSNorm stats via Gram-diagonal matmuls; r_tok comes out token-major
  [128,16]; broadcast to feature-major via 16 tiny DMAs + one gpsimd
  partition broadcast (r_q rides the cos/sin tables, r_k rides the exp
  scale, r_v rides the V psum->sbuf copy).
- AV matmul transposed: exp blocks [keys, 128 queries] are stationary, V
  [keys, 68] streams (col 64 = ones column -> softmax denominator, 65-67
  pad for aligned psum offsets). Token-major avT is normalized with
  per-partition scalars, PE-transposed back to feature-major (bitcast into
  the freed acc psum banks), then out-projected per panel.
- Head-pair staging: attention for heads {0,1} starts right after their
  projections; projections + rope for heads {2,3} are emitted interleaved
  into the pair-0 attention stream so PE fills the exp-bound window.
- Causal attention panel-major (4 panels x 512 queries) per pair, exact
  128-block causal starts, software-pipelined emission (avT lags 2 steps,
  deferred close/out-proj ops drain during subsequent steps).
"""

import os
from collections import deque

import numpy as np
import ml_dtypes

BF16 = ml_dtypes.bfloat16

DIM = 1024
HEADS = 16
DIM_HEAD = 64
T = 2048  # tokens per batch
B = 2
HPC = 4  # heads per core
F = HPC * DIM_HEAD  # 256 per-core head width
KC = DIM // 128  # 8 contraction chunks

_NC_CACHE = {}


def _build_nc():
    import concourse.bacc as bacc
    import concourse.mybir as mybir
    import concourse.tile as tile
    from contextlib import ExitStack

    f32 = mybir.dt.float32
    bf16 = mybir.dt.bfloat16
    nc = bacc.Bacc()

    xT = nc.declare_dram_parameter("xT", [DIM, T], bf16, isOutput=False)
    wq = nc.declare_dram_parameter("wq", [DIM, F], bf16, isOutput=False)
    wk = nc.declare_dram_parameter("wk", [DIM, F], bf16, isOutput=False)
    wv = nc.declare_dram_parameter("wv", [DIM, F], bf16, isOutput=False)
    wo = nc.declare_dram_parameter("wo", [F, DIM], bf16, isOutput=False)
    cosT = nc.declare_dram_parameter("cosT", [128, T], bf16, isOutput=False)
    sinT = nc.declare_dram_parameter("sinT", [128, T], bf16, isOutput=False)
    perm = nc.declare_dram_parameter("perm", [128, 128], bf16, isOutput=False)
    masks = nc.declare_dram_parameter("masks", [128, 128], bf16, isOutput=False)
    ident = nc.declare_dram_parameter("ident", [128, 128], bf16, isOutput=False)
    out = nc.declare_dram_parameter("out", [DIM, T], bf16, isOutput=True)

    Exp = mybir.ActivationFunctionType.Exp
    Sqrt = mybir.ActivationFunctionType.Sqrt
    Copy = mybir.ActivationFunctionType.Copy
    mult = mybir.AluOpType.mult
    add = mybir.AluOpType.add

    with ExitStack() as ctx:
        tc = ctx.enter_context(tile.TileContext(nc))
        consts = ctx.enter_context(tc.tile_pool(name="consts", bufs=1))
        persist = ctx.enter_context(tc.tile_pool(name="persist", bufs=1))
        work = ctx.enter_context(tc.tile_pool(name="work", bufs=6))
        expool = ctx.enter_context(tc.tile_pool(name="expool", bufs=24))
        # [128,512]-f32 psum users (proj/gram/out-proj) share one rotating
        # pool; scores and avT accumulators get their own pools
        ps512 = ctx.enter_context(tc.tile_pool(name="ps512", bufs=3, space="PSUM"))
        psSc = ctx.enter_context(tc.tile_pool(name="psSc", bufs=3, space="PSUM"))
        psAcc = ctx.enter_context(tc.tile_pool(name="psAcc", bufs=2, space="PSUM"))

        # ---- load constants ----
        wq_sb = consts.tile([128, KC, F], bf16, tag="wq")
        wk_sb = consts.tile([128, KC, F], bf16, tag="wk")
        wv_sb = consts.tile([128, KC, F], bf16, tag="wv")
        wo_sb = consts.tile([128, 2, DIM], bf16, tag="wo")
        cos_sb = consts.tile([128, T], bf16, tag="cos")
        sin_sb = consts.tile([128, T], bf16, tag="sin")
        perm_sb = consts.tile([128, 128], bf16, tag="perm")
        mask_sb = consts.tile([128, 128], bf16, tag="mask")
        id_sb = consts.tile([128, 128], bf16, tag="ident")
        xT_sb = persist.tile([128, KC, T], bf16, tag="xT")
        xT_r = xT.rearrange("(kc p) t -> p kc t", p=128)
        for kc in range(KC):
            nc.sync.dma_start(xT_sb[:, kc, 0:512], xT_r[:, kc, 0:512])
        nc.sync.dma_start(id_sb, ident[:, :])
        nc.sync.dma_start(wk_sb, wk.rearrange("(kc p) f -> p kc f", p=128))
        nc.sync.dma_start(cos_sb, cosT[:, :])
        nc.sync.dma_start(sin_sb, sinT[:, :])
        nc.sync.dma_start(perm_sb, perm[:, :])
        nc.sync.dma_start(wq_sb, wq.rearrange("(kc p) f -> p kc f", p=128))
        nc.sync.dma_start(wv_sb, wv.rearrange("(kc p) f -> p kc f", p=128))
        for kc in range(KC):
            nc.sync.dma_start(xT_sb[:, kc, 512:1024], xT_r[:, kc, 512:1024])
        nc.sync.dma_start(mask_sb, masks[:, :])
        nc.sync.dma_start(wo_sb, wo.rearrange("(fc p) d -> p fc d", p=128))
        for kc in range(KC):
            nc.sync.dma_start(xT_sb[:, kc, 1024:2048], xT_r[:, kc, 1024:2048])

        # persistent activation tensors
        qk_sb = persist.tile([128, 4, T], bf16, tag="qk")
        v_sb = persist.tile([128, 16, HPC, 68], bf16, tag="v")
        nc.vector.memset(v_sb[:, :, :, 64:68], 0.0)
        nc.vector.memset(v_sb[:, :, :, 64:65], 1.0)
        cosr_sb = persist.tile([128, T], bf16, tag="cosr")
        sinr_sb = persist.tile([128, T], bf16, tag="sinr")
        r_bc = persist.tile([128, T], f32, tag="rbc")
        ssq = persist.tile([128, 16], f32, tag="ssq")
        r_tok = persist.tile([128, 16], f32, tag="rtok")
        r_row = persist.tile([1, T], f32, tag="rrow")
        dg_scr = persist.tile([128, 128], f32, tag="dgscr")
        avn = persist.tile([128, 16, F], bf16, tag="avn")
        av_fm = persist.tile([128, 2, T], bf16, tag="avfm")
        rinv = persist.tile([128, 16, HPC], f32, tag="rinv")
        sq_tok = persist.tile([128, 16], f32, tag="sqtok")

        # ---- rmsnorm stat units: gram-diag per token block, r-chain per
        # 1024-column half (lets attention start on half 0 early) ----
        def gram_unit(tb):
            g_ps = ps512.tile([128, 512], f32, tag="u512", name=f"g_{tb}")
            cols = slice(tb * 128, (tb + 1) * 128)
            for kc in range(KC):
                nc.tensor.matmul(
                    g_ps[:, 0:128],
                    lhsT=xT_sb[:, kc, cols],
                    rhs=xT_sb[:, kc, cols],
                    start=(kc == 0),
                    stop=(kc == KC - 1),
                )
            nc.vector.tensor_mul(dg_scr, g_ps[:, 0:128], id_sb)
            nc.vector.reduce_sum(
                ssq[:, tb : tb + 1], dg_scr, axis=mybir.AxisListType.X
            )

        def rchain_unit(q4):
            tbs = slice(q4 * 4, q4 * 4 + 4)
            cols = slice(q4 * 512, q4 * 512 + 512)
            nc.scalar.activation(
                sq_tok[:, tbs], ssq[:, tbs], Sqrt, scale=1.0 / DIM
            )
            nc.vector.reciprocal(r_tok[:, tbs], sq_tok[:, tbs])
            for tb in range(q4 * 4, q4 * 4 + 4):
                nc.sync.dma_start(
                    r_row[0:1, tb * 128 : (tb + 1) * 128], r_tok[:, tb : tb + 1]
                )
            nc.gpsimd.partition_broadcast(r_bc[:, cols], r_row[:, cols])
            nc.vector.tensor_tensor(cosr_sb[:, cols], cos_sb[:, cols], r_bc[:, cols], mult)
            nc.vector.tensor_tensor(sinr_sb[:, cols], sin_sb[:, cols], r_bc[:, cols], mult)

        # ---- projection emission units ----
        w_of = {0: (wq_sb, 0), 1: (wq_sb, 1), 2: (wk_sb, 0), 3: (wk_sb, 1)}

        def proj_qk_unit(fidx, tt):
            """One [128,512] projection chunk + rope for q/k head-pair fidx."""
            ts = slice(tt * 512, (tt + 1) * 512)
            wsb, fc = w_of[fidx]
            ps = ps512.tile([128, 512], f32, tag="u512", name=f"p_{fidx}_{tt}")
            for kc in range(KC):
                nc.tensor.matmul(
                    ps,
                    lhsT=wsb[:, kc, fc * 128 : (fc + 1) * 128],
                    rhs=xT_sb[:, kc, ts],
                    start=(kc == 0),
                    stop=(kc == KC - 1),
                )
            raw = work.tile([128, 512], bf16, tag="raw", name=f"raw_{fidx}_{tt}")
            nc.vector.tensor_copy(out=raw, in_=ps)
            cc = cosr_sb if fidx < 2 else cos_sb
            ssb = sinr_sb if fidx < 2 else sin_sb
            pp = ps512.tile([128, 512], f32, tag="u512", name=f"pp_{fidx}_{tt}")
            nc.tensor.matmul(pp, lhsT=perm_sb, rhs=raw, start=True, stop=True)
            t1 = work.tile([128, 512], bf16, tag="ropet1")
            nc.vector.tensor_tensor(t1, pp, ssb[:, ts], mult)
            t2 = work.tile([128, 512], bf16, tag="ropet2")
            nc.vector.tensor_tensor(t2, raw, cc[:, ts], mult)
            nc.gpsimd.tensor_tensor(qk_sb[:, fidx, ts], t2, t1, add)

        def proj_v_unit(tb, pair):
            """V projection for one token block, one head pair (128 cols)."""
            psv = ps512.tile([128, 512], f32, tag="u512", name=f"v_{tb}_{pair}")
            for kc in range(KC):
                nc.tensor.matmul(
                    psv[:, 0:128],
                    lhsT=xT_sb[:, kc, tb * 128 : (tb + 1) * 128],
                    rhs=wv_sb[:, kc, pair * 128 : (pair + 1) * 128],
                    start=(kc == 0),
                    stop=(kc == KC - 1),
                )
            nc.scalar.activation(
                out=v_sb[:, tb, 2 * pair : 2 * pair + 2, 0:64],
                in_=psv[:, 0:128].rearrange("p (h d) -> p h d", h=2),
                func=Copy,
                scale=r_tok[:, tb : tb + 1],
            )

        # startup wavefront: only what pair-0 panel-0 needs is emitted
        # directly (stats/projections for query columns 0-511); everything
        # else becomes units drained into the attention stream, ordered so
        # half-1-dependent units come after the xT half-1 DMA lands.
        for tb in range(4):
            gram_unit(tb)
        rchain_unit(0)
        proj_qk_unit(2, 0)
        proj_qk_unit(0, 0)
        for tb in range(4):
            proj_v_unit(tb, 0)

        units = deque()
        units.append(lambda: proj_qk_unit(2, 1))
        for tb in range(4, 8):
            units.append(lambda tb=tb: gram_unit(tb))
        units.append(lambda: rchain_unit(1))
        units.append(lambda: proj_qk_unit(0, 1))
        for tb in range(4, 8):
            units.append(lambda tb=tb: proj_v_unit(tb, 0))
        for tb in range(8, 16):
            units.append(lambda tb=tb: gram_unit(tb))
        units.append(lambda: rchain_unit(2))
        units.append(lambda: rchain_unit(3))
        units.append(lambda: proj_qk_unit(0, 2))
        units.append(lambda: proj_qk_unit(2, 2))
        for tb in range(8, 12):
            units.append(lambda tb=tb: proj_v_unit(tb, 0))
        units.append(lambda: proj_qk_unit(3, 0))
        units.append(lambda: proj_qk_unit(3, 1))
        for tb in range(4):
            units.append(lambda tb=tb: proj_v_unit(tb, 1))
        units.append(lambda: proj_qk_unit(1, 0))
        units.append(lambda: proj_qk_unit(0, 3))
        units.append(lambda: proj_qk_unit(2, 3))
        for tb in range(12, 16):
            units.append(lambda tb=tb: proj_v_unit(tb, 0))
        for tb in range(4, 8):
            units.append(lambda tb=tb: proj_v_unit(tb, 1))
        units.append(lambda: proj_qk_unit(1, 1))
        units.append(lambda: proj_qk_unit(3, 2))
        units.append(lambda: proj_qk_unit(3, 3))
        for tb in range(8, 16):
            units.append(lambda tb=tb: proj_v_unit(tb, 1))
        units.append(lambda: proj_qk_unit(1, 2))
        units.append(lambda: proj_qk_unit(1, 3))

        # ---- attention: head-pair staged, panel-major ----
        pending = deque()  # lagged avT batches
        closeq = deque()  # deferred close / out-projection ops

        def emit_avt(accs, P, pair, hh, kb, ex):
            def go():
                for ql in range(4):
                    qb = 4 * P + ql
                    if qb < kb:
                        continue
                    nc.tensor.matmul(
                        accs[ql // 2][:, ql % 2, hh],
                        lhsT=ex[:, ql * 128 : (ql + 1) * 128],
                        rhs=v_sb[:, kb, 2 * pair + hh],
                        start=(kb == 0 and hh == 0 and ql % 2 == 0),
                        stop=(hh == 1 and ql % 2 == 1 and kb == qb),
                        skip_group_check=False,
                    )

            return go

        def close_ops_qb(accs, P, pair, ql):
            """Normalize + transpose ops for one query block."""
            ops = []
            if True:
                qb = 4 * P + ql

                def rn(ql=ql, qb=qb):
                    nc.vector.reciprocal(
                        rinv[:, qb, 2 * pair : 2 * pair + 2],
                        accs[ql // 2][:, ql % 2, :, 64],
                    )
                    for hh in range(2):
                        nc.vector.tensor_scalar(
                            avn[
                                :,
                                qb,
                                pair * 128 + hh * 64 : pair * 128 + (hh + 1) * 64,
                            ],
                            accs[ql // 2][:, ql % 2, hh, 0:64],
                            rinv[:, qb, 2 * pair + hh : 2 * pair + hh + 1],
                            None,
                            mult,
                        )

                ops.append(rn)

                def tr(ql=ql, qb=qb):
                    tps = ps512.tile(
                        [128, 512], f32, tag="u512", name=f"tp_{pair}_{qb}"
                    )
                    tp = tps[:, 0:64].bitcast(bf16)
                    nc.tensor.transpose(
                        tp, avn[:, qb, pair * 128 : (pair + 1) * 128], id_sb
                    )
                    nc.vector.tensor_copy(
                        out=av_fm[:, pair, qb * 128 : (qb + 1) * 128], in_=tp
                    )

                ops.append(tr)
            return ops

        def close_ops(accs, P, pair):
            ops = []
            for ql in range(4):
                ops.extend(close_ops_qb(accs, P, pair, ql))
            if pair == 1:
                ops.extend(close_po(P))
            return ops

        def close_po(P):
            ops = []
            if True:
                for tt in (P,):
                    ts = slice(tt * 512, (tt + 1) * 512)
                    for do in range(8):

                        def oj(do=do, ts=ts, tt=tt):
                            po = ps512.tile(
                                [128, 512], f32, tag="u512", name=f"po_{tt}_{do}"
                            )
                            for fc in range(2):
                                nc.tensor.matmul(
                                    po,
                                    lhsT=wo_sb[:, fc, do * 128 : (do + 1) * 128],
                                    rhs=av_fm[:, fc, ts],
                                    start=(fc == 0),
                                    stop=(fc == 1),
                                )
                            ob = work.tile([128, 512], bf16, tag="ob")
                            nc.vector.tensor_copy(out=ob, in_=po)
                            nc.sync.dma_start(
                                out.rearrange("(do p) t -> p do t", p=128)[
                                    :, do, ts
                                ],
                                ob,
                            )

                        ops.append(oj)
            return ops

        step = 0
        for pair in range(2):
            fq, fk = (0, 2) if pair == 0 else (1, 3)
            for P in range(4):
                qlo = P * 512
                accs = [
                    psAcc.tile(
                        [128, 2, 2, 68], f32, tag="acc", name=f"acc_{pair}_{P}_{i}"
                    )
                    for i in range(2)
                ]
                nkb = 4 * (P + 1)
                for hh in range(2):
                    qt = qk_sb[:, fq]
                    kt = qk_sb[:, fk]
                    rows = slice(hh * 64, hh * 64 + 64)
                    for kb in range(nkb):
                        c0 = max(kb * 128 - qlo, 0)
                        sc = psSc.tile(
                            [128, 512],
                            f32,
                            tag="sc",
                            name=f"sc_{pair}_{P}_{hh}_{kb}",
                        )
                        nc.tensor.matmul(
                            sc[:, c0:512],
                            lhsT=kt[rows, kb * 128 : (kb + 1) * 128],
                            rhs=qt[rows, qlo + c0 : qlo + 512],
                            start=True,
                            stop=True,
                        )
                        ex = expool.tile([128, 512], bf16, tag="ex")
                        nc.scalar.activation(
                            ex[:, c0:512],
                            sc[:, c0:512],
                            Exp,
                            scale=r_tok[:, kb : kb + 1],
                        )
                        if kb >= 4 * P:
                            nc.gpsimd.tensor_tensor(
                                ex[:, c0 : c0 + 128],
                                ex[:, c0 : c0 + 128],
                                mask_sb,
                                mult,
                            )
                        pending.append(emit_avt(accs, P, pair, hh, kb, ex))
                        step += 1
                        if closeq:
                            closeq.popleft()()
                        if units:
                            units.popleft()()
                        lag = 0 if (pair == 1 and P == 3) else 2
                        while len(pending) > lag:
                            pending.popleft()()
                        if (
                            pair == 1
                            and P == 3
                            and hh == 1
                            and kb >= 4 * P
                            and (kb - 4 * P) % 2 == 1
                        ):
                            # bank (kb-4P)//2 group just stopped: close its 2 qb
                            for lq in (kb - 4 * P - 1, kb - 4 * P):
                                for op in close_ops_qb(accs, P, pair, lq):
                                    op()
                while pending:
                    pending.popleft()()
                if not (pair == 1 and P == 3):
                    closeq.extend(close_ops(accs, P, pair))
                else:
                    closeq.extend(close_po(P))
        while units:
            units.popleft()()
        while closeq:
            closeq.popleft()()
    nc.compile()
    return nc


def _host_inputs(x, norm_w, w_qkv, w_o, sin, cos):
    """Build the 8 per-core input maps (all bf16)."""
    n = T
    w_eff = np.asarray(w_qkv, np.float64) * np.asarray(norm_w, np.float64)[:, None]
    sin_n = np.asarray(sin, np.float32)[:n]  # [T, 64]
    cos_n = np.asarray(cos, np.float32)[:n]
    sign = np.concatenate([-np.ones(32, np.float32), np.ones(32, np.float32)])
    cos_tile = np.tile(cos_n.T, (2, 1))  # [128, T]
    sin_tile = np.tile((sin_n * sign[None, :]).T, (2, 1))  # [128, T]
    perm = np.zeros((128, 128), np.float32)
    for m in range(128):
        d = m % 64
        k = m + 32 if d < 32 else m - 32
        perm[k, m] = 1.0
    ident_np = np.eye(128, dtype=np.float32)
    ql = np.arange(128)[None, :]
    key = np.arange(128)[:, None]
    masks = (ql >= key).astype(np.float32)

    in_maps = []
    for c in range(8):
        b, g = c // 4, c % 4
        fs = slice(g * F, (g + 1) * F)
        in_maps.append(
            {
                "xT": np.ascontiguousarray(np.asarray(x, np.float32)[b].T).astype(BF16),
                "wq": (w_eff[:, 0:DIM][:, fs] * (DIM_HEAD ** -0.5)).astype(BF16),
                "wk": w_eff[:, DIM : 2 * DIM][:, fs].astype(BF16),
                "wv": w_eff[:, 2 * DIM : 3 * DIM][:, fs].astype(BF16),
                "wo": np.asarray(w_o, np.float32)[fs, :].astype(BF16),
                "cosT": cos_tile.astype(BF16),
                "sinT": sin_tile.astype(BF16),
                "perm": perm.astype(BF16),
                "masks": masks.astype(BF16),
                "ident": ident_np.astype(BF16),
            }
        )
    return in_maps


def kernel(x, norm_w, w_qkv, w_o, b_o, sin, cos):
    from concourse.bass_utils import run_bass_kernel_spmd

    if "nc" not in _NC_CACHE:
        _NC_CACHE["nc"] = _build_nc()
    nc = _NC_CACHE["nc"]
    in_maps = _host_inputs(x, norm_w, w_qkv, w_o, sin, cos)
    trace = bool(int(os.environ.get("KERNEL_TRACE", "0")))
    res = run_bass_kernel_spmd(nc, in_maps, core_ids=list(range(8)), trace=trace)
    if trace and res.exec_time_ns is not None:
        print(f"HW exec time: {res.exec_time_ns} ns")
    outs = [r["out"].astype(np.float32) for r in res.results]  # [1024, T] fm
    b_o = np.asarray(b_o, np.float32)
    full = np.empty((B, T, DIM), np.float32)
    for b in range(B):
        acc = outs[b * 4] + outs[b * 4 + 1] + outs[b * 4 + 2] + outs[b * 4 + 3]
        full[b] = acc.T + b_o[None, :]
    return full

